# revision 43
# baseline (speedup 1.0000x reference)
"""Trainium2 Bass kernel for nn_MAB (dense transformer block).

Reference (B=32, N=512, D=512, H=8, dh=64):
    q = Q@Wq.T+bq  k = K@Wk.T+bk  v = K@Wv.T+bv
    scores = einsum("bqhd,bkhd->bhqk", q, k) / sqrt(512)
    A = softmax(scores, axis=2)            # over the QUERY axis
    attn = einsum("bhqk,bkhd->bqhd", A, v).reshape(B, N, D)
    out = Q + attn @ Wo.T + bo
    ffn = relu(out @ W1.T + b1) @ W2.T + b2
    return out + ffn

Strategy: data-parallel over batch (8 cores x 4 batches, no collectives).

Matmul engine plan (fp8 = e4m3 with the DoubleRow perf mode, two
128-partition contraction tiles per instruction at 0.5 cycles/row; fp8
matmuls are only ISA-legal as full tiles at PE position (0,0)):
  - q/k/v projections: fp8 DR, plain quantized operands.
  - scores: fp8 DR via a zero-tile trick -- each head's 64 features are
    relocated (SBUF->SBUF DMA on the idle DMA queues) to rows 0:64 of a
    per-head chunk of a [128, 9, N] "z" tile whose other rows/chunk-8
    are zeroed once per buffer; DR tile0 contracts the head against
    zero-padded rows and tile1 multiplies the all-zero chunk.
  - Wo: fp8 DR with an extra fp8 residual-correction matmul (plain fp8
    weight quantization alone exceeds the error budget).
  - attention apply + W1/W2: bf16 (fp8 on E/out/h1 is wasteful or too
    lossy; bf16 apply reuses the proven row-packed head-pair layout).
  - residual adds (+Q, +out) ride the PSUM accumulation via identity
    stationary matmuls, removing two elementwise passes.
Weights are pre-scaled x64 on the host so fp8 quantization stays out of
the subnormal range; compensating factors fold into the PSUM->SBUF
conversion ops, and ln(512) is folded into the exp bias so the softmax
normalizer needs no separate scaling pass.

Softmax over the query axis: scores^T tiles ([k, q]) -> ACT exp; the
per-k sums S over q come from the ACT accumulator for most (head,chunk)
tiles and from DVE tensor_reduce for n_dve_chunks of them (engine
balance knob; GPSIMD cannot reduce the free axis or read PSUM at all).
1/S is folded into v (64x fewer elements than E).

Emission is a 4-deep software pipeline with closure-level interleaving:
proj(b) | scores+exp(b-1) (+1/S+v-fold tail) | apply+ffn(b-2), with the
per-head scores+exp emissions round-robined against the other stages'
chunks so the in-order PE queue never camps on the ACT-bound softmax
chain.  PSUM: psA [128,2,512]x2 (proj/ffn/apply), psS1 [128,512]x4
(scores->exp ping-pong).
"""

import math
import os
import sys

import numpy as np
import ml_dtypes

sys.path.insert(0, "/opt/trn_rl_repo")

import concourse.bass as bass  # noqa: E402
import concourse.tile as tile  # noqa: E402
from concourse import bacc  # noqa: E402
from concourse import mybir  # noqa: E402
from concourse.bass_utils import run_bass_kernel_spmd  # noqa: E402

F32 = mybir.dt.float32
BF16 = mybir.dt.bfloat16
F8 = mybir.dt.float8e4
AF = mybir.ActivationFunctionType
ALU = mybir.AluOpType
DR = mybir.MatmulPerfMode.DoubleRow

NP_F8 = ml_dtypes.float8_e4m3
NP_BF = ml_dtypes.bfloat16

B, N, D, H = 32, 512, 512, 8
DH = D // H
NCORES = 8
BLOC = B // NCORES
SCALE = 1.0 / math.sqrt(512.0)
P = 128
KC = D // P  # 4 contraction chunks
MC = D // P  # 4 output-feature chunks
SW = 64.0  # host-side weight pre-scale before fp8 quantization
_NEG_LN512 = -math.log(512.0)

_CACHE = {}

# --- engine-balance knobs -------------------------------------------------
# conv engines: 'v' = DVE, 'g' = GPSIMD/Pool, 'a' = ACT (tensor_scalar-able
# conversions only).  reduce_eng maps reduce index 0..len-1 -> engine.
CFG = {
    # heads whose S comes from the ACT accumulator (the rest are batched
    # exps + DVE tensor_reduce; GPSIMD cannot reduce the free axis at all)
    "accum_heads": (0, 1, 2, 3, 4, 5, 6, 7),
    "n_dve_chunks": 10,
    "n_batched_heads": 0,  # of 32 (head,chunk) exps: no-accum + DVE reduce
    "vt_tail_pool": False,  # v-fold chunks 2-3 on the otherwise-idle Pool
    # note: GPSIMD ("g") cannot read PSUM; psum-reading convs must be v/a
    "conv": {
        "qh": "v", "kh": "v", "v": "v", "vt": "v",
        "attnT": "v", "outbf": "v", "h1": "v", "fin": "a",
    },
}


def _q8(x):
    return np.asarray(x, np.float32).astype(NP_F8)


def _scores_perm():
    """Column permutation for Wq/Wk: chunk m holds
    m=0: heads 0-3 feats 0-31 | m=1: heads 4-7 feats 0-31
    m=2: heads 0-3 feats 32-63 | m=3: heads 4-7 feats 32-63
    (each head's two 32-feature halves share partitions 32a..32a+31)."""
    perm = np.empty(D, np.int64)
    for m in range(4):
        for a in range(4):
            for f in range(32):
                h = 4 * (m & 1) + a
                perm[128 * m + 32 * a + f] = 64 * h + 32 * (m >> 1) + f
    return perm


def _build_program(with_bias):
    nc = bacc.Bacc("TRN2", target_bir_lowering=False, debug=False,
                   num_devices=NCORES)

    qt_d = nc.dram_tensor("qtbf", [BLOC, D, N], BF16,
                          kind="ExternalInput").ap()
    id1_d = nc.dram_tensor("ident1", [P, P], BF16, kind="ExternalInput").ap()
    id2k_d = nc.dram_tensor("ident2k", [P, P], BF16,
                            kind="ExternalInput").ap()
    qT8_d = nc.dram_tensor("qT8", [BLOC, D, N], F8, kind="ExternalInput").ap()
    kT8_d = nc.dram_tensor("kT8", [BLOC, D, N], F8, kind="ExternalInput").ap()
    w_d = {}
    for nm in ("wq8", "wk8", "wv8", "wo8", "rwo8"):
        w_d[nm] = nc.dram_tensor(nm, [D, D], F8, kind="ExternalInput").ap()
    for nm in ("w1bf", "w2bf"):
        w_d[nm] = nc.dram_tensor(nm, [D, D], BF16, kind="ExternalInput").ap()
    b_d = {}
    if with_bias:
        for nm in ("bq", "bk", "bv", "bo", "b1", "b2"):
            b_d[nm] = nc.dram_tensor(nm, [D], F32, kind="ExternalInput").ap()
    outT_d = nc.dram_tensor("outT", [BLOC, D, N], F32,
                            kind="ExternalOutput").ap()

    qt_v = qt_d.rearrange("b (o p) t -> b p o t", p=P)
    qT8_v = qT8_d.rearrange("b (o p) t -> b p o t", p=P)
    kT8_v = kT8_d.rearrange("b (o p) t -> b p o t", p=P)
    outT_v = outT_d.rearrange("b (o p) t -> b p o t", p=P)
    w_v = {k: v.rearrange("(o p) n -> p o n", p=P) for k, v in w_d.items()}
    b_v = {k: v.rearrange("(o p) -> p o", p=P) for k, v in b_d.items()}

    accum_heads = set(CFG["accum_heads"])

    with tile.TileContext(nc) as tc:
        with (
            tc.tile_pool(name="weights", bufs=1) as wpool,
            tc.tile_pool(name="qt", bufs=3) as qt_pool,
            tc.tile_pool(name="in8", bufs=2) as in8_pool,
            tc.tile_pool(name="proj", bufs=3) as proj_pool,
            tc.tile_pool(name="z8", bufs=2) as z8_pool,
            tc.tile_pool(name="e8", bufs=17) as e8_pool,
            tc.tile_pool(name="rsum", bufs=2) as rsum_pool,
            tc.tile_pool(name="attn", bufs=2) as attn_pool,
            tc.tile_pool(name="ffn", bufs=2) as ffn_pool,
            tc.tile_pool(name="fin", bufs=4) as fin_pool,
            tc.tile_pool(name="psA", bufs=2, space="PSUM") as psA,
            tc.tile_pool(name="psS1", bufs=4, space="PSUM") as psS1,
        ):
            def eng(key):
                return {"v": nc.vector, "g": nc.gpsimd, "a": nc.scalar}[key]

            def conv(key, out, in0, scale, relu=False, e=None):
                """out = [relu](in0 * scale) on the configured engine."""
                e = e or CFG["conv"][key]
                if e == "a":
                    nc.scalar.activation(
                        out=out, in_=in0,
                        func=AF.Relu if relu else AF.Identity, scale=scale)
                elif relu:
                    eng(e).tensor_scalar(out=out, in0=in0, scalar1=scale,
                                         scalar2=0.0, op0=ALU.mult,
                                         op1=ALU.max)
                else:
                    eng(e).tensor_scalar(out=out, in0=in0, scalar1=scale,
                                         scalar2=None, op0=ALU.mult)

            # ---- resident weights ---------------------------------------
            w_sb = {}
            for nm in ("wq8", "wk8", "wv8", "wo8", "rwo8"):
                w_sb[nm] = wpool.tile([P, KC, D], F8, tag=f"w_{nm}",
                                      name=f"w_{nm}")
            for nm in ("w1bf", "w2bf"):
                w_sb[nm] = wpool.tile([P, KC, D], BF16, tag=f"w_{nm}",
                                      name=f"w_{nm}")
            qt0 = qt_pool.tile([P, KC, N], BF16, tag="qt", name="qt0")
            nln512 = wpool.tile([P, 1], F32, tag="nln512")
            nc.vector.memset(nln512[:], _NEG_LN512)
            id1 = wpool.tile([P, P], BF16, tag="id1")
            id2k = wpool.tile([P, P], BF16, tag="id2k")
            nc.sync.dma_start(out=id1[:], in_=id1_d)
            nc.sync.dma_start(out=id2k[:], in_=id2k_d)
            q80 = in8_pool.tile([P, KC, N], F8, tag="q8", name="q80")
            k80 = in8_pool.tile([P, KC, N], F8, tag="k8", name="k80")
            # initial loads fan out across four DGE queues so the first
            # projection's operands arrive as early as possible
            nc.sync.dma_start(out=w_sb["wq8"][:], in_=w_v["wq8"])
            nc.gpsimd.dma_start(out=q80[:], in_=qT8_v[0])
            nc.scalar.dma_start(out=w_sb["wk8"][:], in_=w_v["wk8"])
            nc.gpsimd.dma_start(out=k80[:], in_=kT8_v[0])
            nc.gpsimd.dma_start(out=w_sb["wv8"][:], in_=w_v["wv8"])
            nc.sync.dma_start(out=qt0[:], in_=qt_v[0])
            # PE warm-up: harmless matmuls on the identity tile keep the
            # p-state ramp running while the loads land
            wu = psS1.tile([P, N], F32, tag="psS1", name="warmup")
            for _ in range(10):
                nc.tensor.matmul(wu[:, 0:P], lhsT=id1[:], rhs=id1[:],
                                 start=True, stop=True)

            b_sb = {}
            bv_bc = None
            if with_bias:
                for nm in ("bq", "bk", "bo", "b1", "b2"):
                    b_sb[nm] = wpool.tile([P, MC], F32, tag=f"b_{nm}",
                                          name=f"b_{nm}")
                    nc.sync.dma_start(out=b_sb[nm][:], in_=b_v[nm])
                bv_bc = wpool.tile([P, D], F32, tag="bv_bc")
                bv_src = bass.AP(tensor=b_d["bv"].tensor,
                                 offset=b_d["bv"].offset,
                                 ap=[[0, P], *b_d["bv"].ap])
                nc.sync.dma_start(out=bv_bc[:], in_=bv_src)

            st = {}

            def dma_in(b):
                qt_b = qt_pool.tile([P, KC, N], BF16, tag="qt")
                q8_b = in8_pool.tile([P, KC, N], F8, tag="q8")
                k8_b = in8_pool.tile([P, KC, N], F8, tag="k8")
                st[b] = {"qt": qt_b, "q8": q8_b, "k8": k8_b}

                def go():
                    nc.sync.dma_start(out=qt_b[:], in_=qt_v[b])
                    nc.sync.dma_start(out=q8_b[:], in_=qT8_v[b])
                    nc.sync.dma_start(out=k8_b[:], in_=kT8_v[b])
                return [go]

            def proj_closures(b):
                qt_b, q8_b, k8_b = st[b]["qt"], st[b]["q8"], st[b]["k8"]
                qhb = proj_pool.tile([P, MC, N], F8, tag="qhb")
                khb = proj_pool.tile([P, MC, N], F8, tag="khb")
                v_bf = proj_pool.tile([P, KC, D], BF16, tag="v")
                qz = z8_pool.tile([P, 9, N], F8, tag="qz", name=f"qz_{b}")
                kz = z8_pool.tile([P, 9, N], F8, tag="kz", name=f"kz_{b}")
                st[b].update({"qhb": qhb, "khb": khb, "v": v_bf,
                              "qz": qz, "kz": kz})

                def lin_u(dst, key, wname, rhs, bias, u):
                    def go():
                        ps = psA.tile([P, 2, N], F32, tag="psA")
                        for ml in range(2):
                            m = 2 * u + ml
                            for t in range(2):
                                nc.tensor.matmul(
                                    ps[:, ml, :],
                                    lhsT=w_sb[wname][:, 2 * t:2 * t + 2,
                                                     m * P:(m + 1) * P],
                                    rhs=rhs[:, 2 * t:2 * t + 2, :],
                                    start=(t == 0), stop=(t == 1),
                                    perf_mode=DR)
                        if with_bias and bias is not None:
                            for ml in range(2):
                                m = 2 * u + ml
                                eng(CFG["conv"][key]).tensor_scalar(
                                    out=dst[:, m, :], in0=ps[:, ml, :],
                                    scalar1=1.0 / SW,
                                    scalar2=b_sb[bias][:, m:m + 1],
                                    op0=ALU.mult, op1=ALU.add)
                        else:
                            conv(key, dst[:, 2 * u:2 * u + 2, :], ps[:],
                                 1.0 / SW)
                    return go

                def v_u(u):
                    def go():
                        ps = psA.tile([P, 2, N], F32, tag="psA")
                        for tl in range(2):
                            tt = 2 * u + tl
                            for t in range(2):
                                nc.tensor.matmul(
                                    ps[:, tl, :],
                                    lhsT=k8_b[:, 2 * t:2 * t + 2,
                                              tt * P:(tt + 1) * P],
                                    rhs=w_sb["wv8"][:, 2 * t:2 * t + 2, :],
                                    start=(t == 0), stop=(t == 1),
                                    perf_mode=DR)
                        if with_bias:
                            for tl in range(2):
                                tt = 2 * u + tl
                                eng(CFG["conv"]["v"]).scalar_tensor_tensor(
                                    out=v_bf[:, tt, :], in0=ps[:, tl, :],
                                    scalar=1.0 / SW, in1=bv_bc[:],
                                    op0=ALU.mult, op1=ALU.add)
                        else:
                            conv("v", v_bf[:, 2 * u:2 * u + 2, :], ps[:],
                                 1.0 / SW)
                    return go

                def zbuild(u):
                    # per-head relocation: head h's 64 q/k features ->
                    # rows 0:64 of z-chunk h; chunk 8 and rows 64:128 of
                    # qz stay zero (memset once per pool buffer) so the
                    # DoubleRow zero-tile trick contracts only head h.
                    # u=0 covers heads 0-3 (proj chunks 0-1), u=1 heads 4-7.
                    def go():
                        if b < 2 and u == 0:
                            nc.gpsimd.memset(qz[64:P, :, :], 0.0)
                            nc.gpsimd.memset(qz[0:64, 8, :], 0.0)
                            nc.gpsimd.memset(kz[64:P, :, :], 0.0)
                            nc.gpsimd.memset(kz[0:64, 8, :], 0.0)
                        cs = slice(2 * u, 2 * u + 2)
                        for src_t, dst_t in ((qhb, qz), (khb, kz)):
                            nc.sync.dma_start(
                                out=dst_t[0:64, 4 * u:4 * u + 4:2, :],
                                in_=src_t[0:64, cs, :])
                            nc.sync.dma_start(
                                out=dst_t[0:64, 4 * u + 1:4 * u + 4:2, :],
                                in_=src_t[64:P, cs, :])
                    return go

                out = [lin_u(qhb, "qh", "wq8", q8_b, "bq", 0),
                       lin_u(khb, "kh", "wk8", k8_b, "bk", 0),
                       zbuild(0),
                       lin_u(qhb, "qh", "wq8", q8_b, "bq", 1),
                       lin_u(khb, "kh", "wk8", k8_b, "bk", 1),
                       zbuild(1),
                       v_u(0), v_u(1)]
                if b == 0:
                    def defer_w():
                        for nm in ("wo8", "rwo8", "w1bf", "w2bf"):
                            nc.sync.dma_start(out=w_sb[nm][:], in_=w_v[nm])
                    out.append(defer_w)
                return out

            def head_sel(h):
                # head h's 64 features: partitions 64*(h%2).. of chunk h//2
                return slice(64 * (h % 2), 64 * (h % 2) + 64), h // 2

            def scores_prep(b):
                st[b]["racc"] = rsum_pool.tile([P, H, KC], F32, tag="racc",
                                               name=f"racc_{b}")
                st[b]["rrec"] = rsum_pool.tile([P, H, KC], F32, tag="rrec",
                                               name=f"rrec_{b}")
                st[b]["e"] = {}

            def scores_exp(b, h, js):
                qz, kz, racc = st[b]["qz"], st[b]["kz"], st[b]["racc"]
                if js[0] == 0:
                    e_h = e8_pool.tile([P, KC, N], BF16, tag="e8",
                                       name=f"e_{b}_{h}")
                    st[b]["e"][h] = e_h
                else:
                    e_h = st[b]["e"][h]
                zsl = slice(h, 9, 8 - h)  # chunks {h, 8}; 8 is all-zero in qz
                if h < CFG["n_batched_heads"]:
                    # S on DVE: batch the exps [128,1024] to amortize init
                    for cp in (js[0] // 2, js[2] // 2) if len(js) == 4 \
                            else (js[0] // 2,):
                        ps = psA.tile([P, 2, N], F32, tag="psA")
                        for jj in range(2):
                            j = 2 * cp + jj
                            nc.tensor.matmul(
                                ps[:, jj, :],
                                lhsT=kz[:, zsl, j * P:(j + 1) * P],
                                rhs=qz[:, zsl, :], start=True, stop=True,
                                perf_mode=DR)
                        nc.scalar.activation(
                            out=e_h[:, 2 * cp:2 * cp + 2, :], in_=ps[:],
                            func=AF.Exp, scale=SCALE, bias=nln512[:])
                        for jj in range(2):
                            j = 2 * cp + jj
                            nc.vector.tensor_reduce(
                                out=racc[:, h, j:j + 1], in_=e_h[:, j, :],
                                axis=mybir.AxisListType.X, op=ALU.add)
                    return
                for j in js:
                    ps = psS1.tile([P, N], F32, tag="psS1")
                    nc.tensor.matmul(
                        ps[:], lhsT=kz[:, zsl, j * P:(j + 1) * P],
                        rhs=qz[:, zsl, :], start=True, stop=True,
                        perf_mode=DR)
                    if h * KC + j < CFG["n_dve_chunks"]:
                        nc.scalar.activation(
                            out=e_h[:, j, :], in_=ps[:],
                            func=AF.Exp, scale=SCALE, bias=nln512[:])
                        nc.vector.tensor_reduce(
                            out=racc[:, h, j:j + 1], in_=e_h[:, j, :],
                            axis=mybir.AxisListType.X, op=ALU.add)
                    else:
                        nc.scalar.activation(
                            out=e_h[:, j, :], in_=ps[:],
                            func=AF.Exp, scale=SCALE, bias=nln512[:],
                            accum_out=racc[:, h, j:j + 1])

            def vt_closures(b):
                racc, rrec, v_bf = st[b]["racc"], st[b]["rrec"], st[b]["v"]
                vtb = attn_pool.tile([P, KC, D], BF16, tag="vtb")
                st[b]["vtb"] = vtb

                def per_tt(tt):
                    def go():
                        # racc holds S/512 (ln512 folded into the exp bias),
                        # so rrec = 512/S directly
                        nc.vector.reciprocal(out=rrec[:, :, tt],
                                             in_=racc[:, :, tt])
                        base = rrec[:, 0, tt]
                        r_bc = bass.AP(tensor=base.tensor, offset=base.offset,
                                       ap=[base.ap[0], [KC, H], [0, DH]])
                        vsl = v_bf[:, tt, :]
                        v3 = bass.AP(tensor=vsl.tensor, offset=vsl.offset,
                                     ap=[vsl.ap[0], [DH, H], [1, DH]])
                        osl = vtb[:, tt, :]
                        o3 = bass.AP(tensor=osl.tensor, offset=osl.offset,
                                     ap=[osl.ap[0], [DH, H], [1, DH]])
                        if tt >= 2 and CFG["vt_tail_pool"]:
                            nc.gpsimd.tensor_tensor(out=o3, in0=v3, in1=r_bc,
                                                    op=ALU.mult)
                        else:
                            eng(CFG["conv"]["vt"]).tensor_tensor(
                                out=o3, in0=v3, in1=r_bc, op=ALU.mult)
                    return go
                return [per_tt(tt) for tt in range(KC)]

            def apply_closures(b):
                e_t = st[b]["e"]
                attnT8 = attn_pool.tile([P, MC, N], F8, tag="attnT8")
                st[b]["attnT8"] = attnT8

                def per_u(u):
                    def go():
                        vtb = st[b]["vtb"]
                        ps = psA.tile([P, 2, N], F32, tag="psA")
                        for hl in range(2):
                            hp = 2 * u + hl
                            for hh in range(2):
                                h = 2 * hp + hh
                                po = 64 * hh
                                for j in range(KC):
                                    nc.tensor.matmul(
                                        ps[po:po + 64, hl, :],
                                        lhsT=vtb[:, j, 64 * h:64 * h + 64],
                                        rhs=e_t[h][:, j, :],
                                        start=(j == 0), stop=(j == KC - 1),
                                        tile_position=(0, po))
                        conv("attnT", attnT8[:, 2 * u:2 * u + 2, :], ps[:],
                             32.0, e="a" if b == BLOC - 1 else None)
                    return go
                return [per_u(0), per_u(1)]

            def ffn_closures(b):
                qt_b = st[b]["qt"]
                outbf = ffn_pool.tile([P, MC, N], BF16, tag="outbf")
                h1bf = ffn_pool.tile([P, MC, N], BF16, tag="h1bf")

                def wo_u(u):
                    def go():
                        attnT8 = st[b]["attnT8"]
                        ps = psA.tile([P, 2, N], F32, tag="psA")
                        for ml in range(2):
                            m = 2 * u + ml
                            first = True
                            for t in range(2):
                                for wname in ("wo8", "rwo8"):
                                    nc.tensor.matmul(
                                        ps[:, ml, :],
                                        lhsT=w_sb[wname][:, 2 * t:2 * t + 2,
                                                         m * P:(m + 1) * P],
                                        rhs=attnT8[:, 2 * t:2 * t + 2, :],
                                        start=first, stop=False,
                                        perf_mode=DR)
                                    first = False
                            nc.tensor.matmul(
                                ps[:, ml, :], lhsT=id2k[:],
                                rhs=qt_b[:, m, :], start=False, stop=True)
                        if with_bias:
                            for ml in range(2):
                                m = 2 * u + ml
                                eng(CFG["conv"]["outbf"]).tensor_scalar(
                                    out=outbf[:, m, :], in0=ps[:, ml, :],
                                    scalar1=1.0 / (16.0 * SW * 2.0),
                                    scalar2=b_sb["bo"][:, m:m + 1],
                                    op0=ALU.mult, op1=ALU.add)
                        else:
                            conv("outbf", outbf[:, 2 * u:2 * u + 2, :],
                                 ps[:], 1.0 / (16.0 * SW * 2.0),
                                 e="a" if b == BLOC - 1 else None)
                    return go

                def w1_u(u):
                    def go():
                        ps = psA.tile([P, 2, N], F32, tag="psA")
                        for ml in range(2):
                            m = 2 * u + ml
                            for kc in range(KC):
                                nc.tensor.matmul(
                                    ps[:, ml, :],
                                    lhsT=w_sb["w1bf"][:, kc,
                                                      m * P:(m + 1) * P],
                                    rhs=outbf[:, kc, :],
                                    start=(kc == 0), stop=(kc == KC - 1))
                        if with_bias:
                            for ml in range(2):
                                m = 2 * u + ml
                                eng(CFG["conv"]["h1"]).tensor_scalar(
                                    out=h1bf[:, m, :], in0=ps[:, ml, :],
                                    scalar1=b_sb["b1"][:, m:m + 1],
                                    scalar2=0.0, op0=ALU.add, op1=ALU.max)
                        else:
                            conv("h1", h1bf[:, 2 * u:2 * u + 2, :], ps[:],
                                 1.0, relu=True,
                                 e="a" if b == BLOC - 1 else None)
                    return go

                def w2_u(u):
                    def go():
                        ps = psA.tile([P, 2, N], F32, tag="psA")
                        for ml in range(2):
                            m = 2 * u + ml
                            for kc in range(KC):
                                nc.tensor.matmul(
                                    ps[:, ml, :],
                                    lhsT=w_sb["w2bf"][:, kc,
                                                      m * P:(m + 1) * P],
                                    rhs=h1bf[:, kc, :],
                                    start=(kc == 0), stop=False)
                            nc.tensor.matmul(
                                ps[:, ml, :], lhsT=id1[:],
                                rhs=outbf[:, m, :], start=False, stop=True)
                        fin = fin_pool.tile([P, 2, N], F32, tag="fin")
                        if with_bias:
                            for ml in range(2):
                                m = 2 * u + ml
                                nc.vector.tensor_scalar(
                                    out=fin[:, ml, :], in0=ps[:, ml, :],
                                    scalar1=b_sb["b2"][:, m:m + 1],
                                    scalar2=None, op0=ALU.add)
                        else:
                            conv("fin", fin[:], ps[:], 1.0)
                        nc.sync.dma_start(
                            out=outT_v[b][:, 2 * u:2 * u + 2, :], in_=fin[:])
                    return go

                return [wo_u(0), wo_u(1), w1_u(0), w1_u(1), w2_u(0), w2_u(1)]

            # ---- interleaved software pipeline --------------------------
            # stage s: proj(s) | scores+exp(s-1) | vt/apply/ffn(s-2).
            # Per-head scores+exp emissions are round-robined with the other
            # stages' chunks so the in-order PE queue never camps on the
            # ACT-bound softmax chain.
            for b in range(1):
                pass
            # batch 0 inputs load up-front (qt0/q80/k80 already DMA'd)
            st[0] = {"qt": qt0, "q8": q80, "k8": k80}
            deferred = []
            for s in range(BLOC + 2):
                others = list(deferred)
                deferred = []
                if s < BLOC and s > 0:
                    others += dma_in(s)
                if s >= 2:
                    others += apply_closures(s - 2)
                    fc = ffn_closures(s - 2)
                    if s == BLOC:
                        # hold back half the ffn so the pipeline drain has
                        # work to overlap with the last batch's apply
                        others += fc[:2]
                        deferred = fc[2:]
                    else:
                        others += fc
                if s < BLOC:
                    others += proj_closures(s)
                if 1 <= s <= BLOC:
                    b_sc = s - 1
                    scores_prep(b_sc)
                    k = 0
                    for i_h in range(H):
                        scores_exp(b_sc, i_h, (0, 1, 2, 3))
                        take = ((len(others) * (i_h + 1)) // H
                                - (len(others) * i_h) // H)
                        for _ in range(take):
                            others[k]()
                            k += 1
                    while k < len(others):
                        others[k]()
                        k += 1
                    # 1/S + v-fold as soon as all sums are in
                    for c in vt_closures(b_sc):
                        c()
                else:
                    for c in others:
                        c()

    nc.compile()
    return nc


def kernel(Q, K, Wq, bq, Wk, bk, Wv, bv, Wo, bo, W1, b1, W2, b2):
    Q = np.asarray(Q, dtype=np.float32)
    K = np.asarray(K, dtype=np.float32)

    biases = {nm: np.asarray(v, np.float32) for nm, v in
              (("bq", bq), ("bk", bk), ("bv", bv),
               ("bo", bo), ("b1", b1), ("b2", b2))}
    with_bias = any(np.any(v) for v in biases.values())

    key = ("nc", with_bias)
    if key not in _CACHE:
        _CACHE[key] = _build_program(with_bias)
    nc = _CACHE[key]

    wqT = np.asarray(Wq, np.float32).T * SW
    wkT = np.asarray(Wk, np.float32).T * SW
    woT = np.asarray(Wo, np.float32).T * SW
    wo8 = _q8(woT)
    common = {
        "wq8": np.ascontiguousarray(_q8(wqT)),
        "wk8": np.ascontiguousarray(_q8(wkT)),
        "wv8": np.ascontiguousarray(_q8(np.asarray(Wv, np.float32).T * SW)),
        "wo8": np.ascontiguousarray(wo8),
        "rwo8": np.ascontiguousarray(_q8(woT - wo8.astype(np.float32))),
        "w1bf": np.ascontiguousarray(
            np.asarray(W1, np.float32).T.astype(NP_BF)),
        "w2bf": np.ascontiguousarray(
            np.asarray(W2, np.float32).T.astype(NP_BF)),
        "ident1": np.eye(P, dtype=np.float32).astype(NP_BF),
        "ident2k": (np.eye(P, dtype=np.float32) * 2048.0).astype(NP_BF),
    }
    if with_bias:
        common.update({
            "bq": biases["bq"], "bk": biases["bk"],
            "bv": biases["bv"], "bo": biases["bo"],
            "b1": biases["b1"], "b2": biases["b2"],
        })
    in_maps = []
    for c in range(NCORES):
        sl = slice(c * BLOC, (c + 1) * BLOC)
        qT = np.ascontiguousarray(Q[sl].transpose(0, 2, 1))
        kT = np.ascontiguousarray(K[sl].transpose(0, 2, 1))
        in_maps.append({
            "qtbf": qT.astype(NP_BF),
            "qT8": _q8(qT),
            "kT8": _q8(kT),
            **common,
        })

    trace = bool(int(os.environ.get("KERNEL_TRACE", "0")))
    res = run_bass_kernel_spmd(nc, in_maps, core_ids=list(range(NCORES)),
                               trace=trace)
    if trace and res.exec_time_ns is not None:
        print(f"HW exec time: {res.exec_time_ns} ns")

    out = np.empty((B, N, D), np.float32)
    for c in range(NCORES):
        out[c * BLOC:(c + 1) * BLOC] = res.results[c]["outT"].transpose(0, 2, 1)
    return out


# revision 44
# speedup vs baseline: 1.0035x; 1.0035x over previous
"""Trainium2 Bass kernel for nn_MAB (dense transformer block).

Reference (B=32, N=512, D=512, H=8, dh=64):
    q = Q@Wq.T+bq  k = K@Wk.T+bk  v = K@Wv.T+bv
    scores = einsum("bqhd,bkhd->bhqk", q, k) / sqrt(512)
    A = softmax(scores, axis=2)            # over the QUERY axis
    attn = einsum("bhqk,bkhd->bqhd", A, v).reshape(B, N, D)
    out = Q + attn @ Wo.T + bo
    ffn = relu(out @ W1.T + b1) @ W2.T + b2
    return out + ffn

Strategy: data-parallel over batch (8 cores x 4 batches, no collectives).

Matmul engine plan (fp8 = e4m3 with the DoubleRow perf mode, two
128-partition contraction tiles per instruction at 0.5 cycles/row; fp8
matmuls are only ISA-legal as full tiles at PE position (0,0)):
  - q/k/v projections: fp8 DR, plain quantized operands.
  - scores: fp8 DR via a zero-tile trick -- each head's 64 features are
    relocated (SBUF->SBUF DMA on the idle DMA queues) to rows 0:64 of a
    per-head chunk of a [128, 9, N] "z" tile whose other rows/chunk-8
    are zeroed once per buffer; DR tile0 contracts the head against
    zero-padded rows and tile1 multiplies the all-zero chunk.
  - Wo: fp8 DR with an extra fp8 residual-correction matmul (plain fp8
    weight quantization alone exceeds the error budget).
  - attention apply + W1/W2: bf16 (fp8 on E/out/h1 is wasteful or too
    lossy; bf16 apply reuses the proven row-packed head-pair layout).
  - residual adds (+Q, +out) ride the PSUM accumulation via identity
    stationary matmuls, removing two elementwise passes.
Weights are pre-scaled x64 on the host so fp8 quantization stays out of
the subnormal range; compensating factors fold into the PSUM->SBUF
conversion ops, and ln(512) is folded into the exp bias so the softmax
normalizer needs no separate scaling pass.

Softmax over the query axis: scores^T tiles ([k, q]) -> ACT exp; the
per-k sums S over q come from the ACT accumulator for most (head,chunk)
tiles and from DVE tensor_reduce for n_dve_chunks of them (engine
balance knob; GPSIMD cannot reduce the free axis or read PSUM at all).
1/S is folded into v (64x fewer elements than E).

Emission is a 4-deep software pipeline with closure-level interleaving:
proj(b) | scores+exp(b-1) (+1/S+v-fold tail) | apply+ffn(b-2), with the
per-head scores+exp emissions round-robined against the other stages'
chunks so the in-order PE queue never camps on the ACT-bound softmax
chain.  PSUM: psA [128,2,512]x2 (proj/ffn/apply), psS1 [128,512]x4
(scores->exp ping-pong).
"""

import math
import os
import sys

import numpy as np
import ml_dtypes

sys.path.insert(0, "/opt/trn_rl_repo")

import concourse.bass as bass  # noqa: E402
import concourse.tile as tile  # noqa: E402
from concourse import bacc  # noqa: E402
from concourse import mybir  # noqa: E402
from concourse.bass_utils import run_bass_kernel_spmd  # noqa: E402

F32 = mybir.dt.float32
BF16 = mybir.dt.bfloat16
F8 = mybir.dt.float8e4
AF = mybir.ActivationFunctionType
ALU = mybir.AluOpType
DR = mybir.MatmulPerfMode.DoubleRow

NP_F8 = ml_dtypes.float8_e4m3
NP_BF = ml_dtypes.bfloat16

B, N, D, H = 32, 512, 512, 8
DH = D // H
NCORES = 8
BLOC = B // NCORES
SCALE = 1.0 / math.sqrt(512.0)
P = 128
KC = D // P  # 4 contraction chunks
MC = D // P  # 4 output-feature chunks
SW = 64.0  # host-side weight pre-scale before fp8 quantization
_NEG_LN512 = -math.log(512.0)

_CACHE = {}

# --- engine-balance knobs -------------------------------------------------
# conv engines: 'v' = DVE, 'g' = GPSIMD/Pool, 'a' = ACT (tensor_scalar-able
# conversions only).  reduce_eng maps reduce index 0..len-1 -> engine.
CFG = {
    # heads whose S comes from the ACT accumulator (the rest are batched
    # exps + DVE tensor_reduce; GPSIMD cannot reduce the free axis at all)
    "accum_heads": (0, 1, 2, 3, 4, 5, 6, 7),
    "n_dve_chunks": 10,
    "n_batched_heads": 0,  # of 32 (head,chunk) exps: no-accum + DVE reduce
    "vt_tail_pool": False,  # v-fold chunks 2-3 on the otherwise-idle Pool
    # note: GPSIMD ("g") cannot read PSUM; psum-reading convs must be v/a
    "conv": {
        "qh": "v", "kh": "v", "v": "v", "vt": "v",
        "attnT": "v", "outbf": "v", "h1": "v", "fin": "a",
    },
}


def _q8(x):
    return np.asarray(x, np.float32).astype(NP_F8)


def _scores_perm():
    """Column permutation for Wq/Wk: chunk m holds
    m=0: heads 0-3 feats 0-31 | m=1: heads 4-7 feats 0-31
    m=2: heads 0-3 feats 32-63 | m=3: heads 4-7 feats 32-63
    (each head's two 32-feature halves share partitions 32a..32a+31)."""
    perm = np.empty(D, np.int64)
    for m in range(4):
        for a in range(4):
            for f in range(32):
                h = 4 * (m & 1) + a
                perm[128 * m + 32 * a + f] = 64 * h + 32 * (m >> 1) + f
    return perm


def _build_program(with_bias):
    nc = bacc.Bacc("TRN2", target_bir_lowering=False, debug=False,
                   num_devices=NCORES)

    qt_d = nc.dram_tensor("qtbf", [BLOC, D, N], BF16,
                          kind="ExternalInput").ap()
    id1_d = nc.dram_tensor("ident1", [P, P], BF16, kind="ExternalInput").ap()
    id2k_d = nc.dram_tensor("ident2k", [P, P], BF16,
                            kind="ExternalInput").ap()
    qT8_d = nc.dram_tensor("qT8", [BLOC, D, N], F8, kind="ExternalInput").ap()
    kT8_d = nc.dram_tensor("kT8", [BLOC, D, N], F8, kind="ExternalInput").ap()
    w_d = {}
    for nm in ("wq8", "wk8", "wv8", "wo8", "rwo8"):
        w_d[nm] = nc.dram_tensor(nm, [D, D], F8, kind="ExternalInput").ap()
    for nm in ("w1bf", "w2bf"):
        w_d[nm] = nc.dram_tensor(nm, [D, D], BF16, kind="ExternalInput").ap()
    b_d = {}
    if with_bias:
        for nm in ("bq", "bk", "bv", "bo", "b1", "b2"):
            b_d[nm] = nc.dram_tensor(nm, [D], F32, kind="ExternalInput").ap()
    outT_d = nc.dram_tensor("outT", [BLOC, D, N], F32,
                            kind="ExternalOutput").ap()

    qt_v = qt_d.rearrange("b (o p) t -> b p o t", p=P)
    qT8_v = qT8_d.rearrange("b (o p) t -> b p o t", p=P)
    kT8_v = kT8_d.rearrange("b (o p) t -> b p o t", p=P)
    outT_v = outT_d.rearrange("b (o p) t -> b p o t", p=P)
    w_v = {k: v.rearrange("(o p) n -> p o n", p=P) for k, v in w_d.items()}
    b_v = {k: v.rearrange("(o p) -> p o", p=P) for k, v in b_d.items()}

    accum_heads = set(CFG["accum_heads"])

    with tile.TileContext(nc) as tc:
        with (
            tc.tile_pool(name="weights", bufs=1) as wpool,
            tc.tile_pool(name="qt", bufs=3) as qt_pool,
            tc.tile_pool(name="in8", bufs=2) as in8_pool,
            tc.tile_pool(name="proj", bufs=3) as proj_pool,
            tc.tile_pool(name="z8", bufs=2) as z8_pool,
            tc.tile_pool(name="e8", bufs=17) as e8_pool,
            tc.tile_pool(name="rsum", bufs=2) as rsum_pool,
            tc.tile_pool(name="attn", bufs=2) as attn_pool,
            tc.tile_pool(name="ffn", bufs=2) as ffn_pool,
            tc.tile_pool(name="fin", bufs=4) as fin_pool,
            tc.tile_pool(name="psA", bufs=2, space="PSUM") as psA,
            tc.tile_pool(name="psS1", bufs=4, space="PSUM") as psS1,
        ):
            def eng(key):
                return {"v": nc.vector, "g": nc.gpsimd, "a": nc.scalar}[key]

            def conv(key, out, in0, scale, relu=False, e=None):
                """out = [relu](in0 * scale) on the configured engine."""
                e = e or CFG["conv"][key]
                if e == "a":
                    nc.scalar.activation(
                        out=out, in_=in0,
                        func=AF.Relu if relu else AF.Identity, scale=scale)
                elif relu:
                    eng(e).tensor_scalar(out=out, in0=in0, scalar1=scale,
                                         scalar2=0.0, op0=ALU.mult,
                                         op1=ALU.max)
                else:
                    eng(e).tensor_scalar(out=out, in0=in0, scalar1=scale,
                                         scalar2=None, op0=ALU.mult)

            # ---- resident weights ---------------------------------------
            w_sb = {}
            for nm in ("wq8", "wk8", "wv8", "wo8", "rwo8"):
                w_sb[nm] = wpool.tile([P, KC, D], F8, tag=f"w_{nm}",
                                      name=f"w_{nm}")
            for nm in ("w1bf", "w2bf"):
                w_sb[nm] = wpool.tile([P, KC, D], BF16, tag=f"w_{nm}",
                                      name=f"w_{nm}")
            qt0 = qt_pool.tile([P, KC, N], BF16, tag="qt", name="qt0")
            nln512 = wpool.tile([P, 1], F32, tag="nln512")
            nc.vector.memset(nln512[:], _NEG_LN512)
            id1 = wpool.tile([P, P], BF16, tag="id1")
            id2k = wpool.tile([P, P], BF16, tag="id2k")
            nc.sync.dma_start(out=id1[:], in_=id1_d)
            nc.sync.dma_start(out=id2k[:], in_=id2k_d)
            q80 = in8_pool.tile([P, KC, N], F8, tag="q8", name="q80")
            k80 = in8_pool.tile([P, KC, N], F8, tag="k8", name="k80")
            # initial loads fan out across four DGE queues so the first
            # projection's operands arrive as early as possible
            nc.sync.dma_start(out=w_sb["wq8"][:], in_=w_v["wq8"])
            nc.gpsimd.dma_start(out=q80[:], in_=qT8_v[0])
            nc.scalar.dma_start(out=w_sb["wk8"][:], in_=w_v["wk8"])
            nc.gpsimd.dma_start(out=k80[:], in_=kT8_v[0])
            nc.gpsimd.dma_start(out=w_sb["wv8"][:], in_=w_v["wv8"])
            nc.sync.dma_start(out=qt0[:], in_=qt_v[0])
            # PE warm-up: harmless matmuls on the identity tile keep the
            # p-state ramp running while the loads land
            wu = psS1.tile([P, N], F32, tag="psS1", name="warmup")
            for _ in range(10):
                nc.tensor.matmul(wu[:, 0:P], lhsT=id1[:], rhs=id1[:],
                                 start=True, stop=True)

            b_sb = {}
            bv_bc = None
            if with_bias:
                for nm in ("bq", "bk", "bo", "b1", "b2"):
                    b_sb[nm] = wpool.tile([P, MC], F32, tag=f"b_{nm}",
                                          name=f"b_{nm}")
                    nc.sync.dma_start(out=b_sb[nm][:], in_=b_v[nm])
                bv_bc = wpool.tile([P, D], F32, tag="bv_bc")
                bv_src = bass.AP(tensor=b_d["bv"].tensor,
                                 offset=b_d["bv"].offset,
                                 ap=[[0, P], *b_d["bv"].ap])
                nc.sync.dma_start(out=bv_bc[:], in_=bv_src)

            st = {}

            def dma_in(b):
                qt_b = qt_pool.tile([P, KC, N], BF16, tag="qt")
                q8_b = in8_pool.tile([P, KC, N], F8, tag="q8")
                k8_b = in8_pool.tile([P, KC, N], F8, tag="k8")
                st[b] = {"qt": qt_b, "q8": q8_b, "k8": k8_b}

                def go():
                    nc.sync.dma_start(out=qt_b[:], in_=qt_v[b])
                    nc.sync.dma_start(out=q8_b[:], in_=qT8_v[b])
                    nc.sync.dma_start(out=k8_b[:], in_=kT8_v[b])
                return [go]

            def proj_closures(b):
                qt_b, q8_b, k8_b = st[b]["qt"], st[b]["q8"], st[b]["k8"]
                qhb = proj_pool.tile([P, MC, N], F8, tag="qhb")
                khb = proj_pool.tile([P, MC, N], F8, tag="khb")
                v_bf = proj_pool.tile([P, KC, D], BF16, tag="v")
                qz = z8_pool.tile([P, 9, N], F8, tag="qz", name=f"qz_{b}")
                kz = z8_pool.tile([P, 9, N], F8, tag="kz", name=f"kz_{b}")
                st[b].update({"qhb": qhb, "khb": khb, "v": v_bf,
                              "qz": qz, "kz": kz})

                def lin_u(dst, key, wname, rhs, bias, u):
                    def go():
                        ps = psA.tile([P, 2, N], F32, tag="psA")
                        for ml in range(2):
                            m = 2 * u + ml
                            for t in range(2):
                                nc.tensor.matmul(
                                    ps[:, ml, :],
                                    lhsT=w_sb[wname][:, 2 * t:2 * t + 2,
                                                     m * P:(m + 1) * P],
                                    rhs=rhs[:, 2 * t:2 * t + 2, :],
                                    start=(t == 0), stop=(t == 1),
                                    perf_mode=DR)
                        if with_bias and bias is not None:
                            for ml in range(2):
                                m = 2 * u + ml
                                eng(CFG["conv"][key]).tensor_scalar(
                                    out=dst[:, m, :], in0=ps[:, ml, :],
                                    scalar1=1.0 / SW,
                                    scalar2=b_sb[bias][:, m:m + 1],
                                    op0=ALU.mult, op1=ALU.add)
                        else:
                            conv(key, dst[:, 2 * u:2 * u + 2, :], ps[:],
                                 1.0 / SW)
                    return go

                def v_u(u):
                    def go():
                        ps = psA.tile([P, 2, N], F32, tag="psA")
                        for tl in range(2):
                            tt = 2 * u + tl
                            for t in range(2):
                                nc.tensor.matmul(
                                    ps[:, tl, :],
                                    lhsT=k8_b[:, 2 * t:2 * t + 2,
                                              tt * P:(tt + 1) * P],
                                    rhs=w_sb["wv8"][:, 2 * t:2 * t + 2, :],
                                    start=(t == 0), stop=(t == 1),
                                    perf_mode=DR)
                        if with_bias:
                            for tl in range(2):
                                tt = 2 * u + tl
                                eng(CFG["conv"]["v"]).scalar_tensor_tensor(
                                    out=v_bf[:, tt, :], in0=ps[:, tl, :],
                                    scalar=1.0 / SW, in1=bv_bc[:],
                                    op0=ALU.mult, op1=ALU.add)
                        else:
                            conv("v", v_bf[:, 2 * u:2 * u + 2, :], ps[:],
                                 1.0 / SW)
                    return go

                def zbuild(u):
                    # per-head relocation: head h's 64 q/k features ->
                    # rows 0:64 of z-chunk h; chunk 8 and rows 64:128 of
                    # qz stay zero (memset once per pool buffer) so the
                    # DoubleRow zero-tile trick contracts only head h.
                    # u=0 covers heads 0-3 (proj chunks 0-1), u=1 heads 4-7.
                    def go():
                        if b < 2 and u == 0:
                            nc.gpsimd.memset(qz[64:P, :, :], 0.0)
                            nc.gpsimd.memset(qz[0:64, 8, :], 0.0)
                            nc.gpsimd.memset(kz[64:P, :, :], 0.0)
                            nc.gpsimd.memset(kz[0:64, 8, :], 0.0)
                        cs = slice(2 * u, 2 * u + 2)
                        for src_t, dst_t in ((qhb, qz), (khb, kz)):
                            nc.sync.dma_start(
                                out=dst_t[0:64, 4 * u:4 * u + 4:2, :],
                                in_=src_t[0:64, cs, :])
                            nc.sync.dma_start(
                                out=dst_t[0:64, 4 * u + 1:4 * u + 4:2, :],
                                in_=src_t[64:P, cs, :])
                    return go

                out = [lin_u(qhb, "qh", "wq8", q8_b, "bq", 0),
                       lin_u(khb, "kh", "wk8", k8_b, "bk", 0),
                       zbuild(0),
                       lin_u(qhb, "qh", "wq8", q8_b, "bq", 1),
                       lin_u(khb, "kh", "wk8", k8_b, "bk", 1),
                       zbuild(1),
                       v_u(0), v_u(1)]
                if b == 0:
                    def defer_w():
                        for nm in ("wo8", "rwo8", "w1bf", "w2bf"):
                            nc.sync.dma_start(out=w_sb[nm][:], in_=w_v[nm])
                    out.append(defer_w)
                return out

            def head_sel(h):
                # head h's 64 features: partitions 64*(h%2).. of chunk h//2
                return slice(64 * (h % 2), 64 * (h % 2) + 64), h // 2

            def scores_prep(b):
                st[b]["racc"] = rsum_pool.tile([P, H, KC], F32, tag="racc",
                                               name=f"racc_{b}")
                st[b]["rrec"] = rsum_pool.tile([P, H, KC], F32, tag="rrec",
                                               name=f"rrec_{b}")
                st[b]["e"] = {}

            def scores_exp(b, h, js):
                qz, kz, racc = st[b]["qz"], st[b]["kz"], st[b]["racc"]
                if js[0] == 0:
                    e_h = e8_pool.tile([P, KC, N], BF16, tag="e8",
                                       name=f"e_{b}_{h}")
                    st[b]["e"][h] = e_h
                else:
                    e_h = st[b]["e"][h]
                zsl = slice(h, 9, 8 - h)  # chunks {h, 8}; 8 is all-zero in qz
                if h < CFG["n_batched_heads"]:
                    # S on DVE: batch the exps [128,1024] to amortize init
                    for cp in (js[0] // 2, js[2] // 2) if len(js) == 4 \
                            else (js[0] // 2,):
                        ps = psA.tile([P, 2, N], F32, tag="psA")
                        for jj in range(2):
                            j = 2 * cp + jj
                            nc.tensor.matmul(
                                ps[:, jj, :],
                                lhsT=kz[:, zsl, j * P:(j + 1) * P],
                                rhs=qz[:, zsl, :], start=True, stop=True,
                                perf_mode=DR)
                        nc.scalar.activation(
                            out=e_h[:, 2 * cp:2 * cp + 2, :], in_=ps[:],
                            func=AF.Exp, scale=SCALE, bias=nln512[:])
                        for jj in range(2):
                            j = 2 * cp + jj
                            nc.vector.tensor_reduce(
                                out=racc[:, h, j:j + 1], in_=e_h[:, j, :],
                                axis=mybir.AxisListType.X, op=ALU.add)
                    return
                for j in js:
                    ps = psS1.tile([P, N], F32, tag="psS1")
                    nc.tensor.matmul(
                        ps[:], lhsT=kz[:, zsl, j * P:(j + 1) * P],
                        rhs=qz[:, zsl, :], start=True, stop=True,
                        perf_mode=DR)
                    if j < 2 and h < 5:  # n_dve placement: early chunks of heads 0-4
                        nc.scalar.activation(
                            out=e_h[:, j, :], in_=ps[:],
                            func=AF.Exp, scale=SCALE, bias=nln512[:])
                        nc.vector.tensor_reduce(
                            out=racc[:, h, j:j + 1], in_=e_h[:, j, :],
                            axis=mybir.AxisListType.X, op=ALU.add)
                    else:
                        nc.scalar.activation(
                            out=e_h[:, j, :], in_=ps[:],
                            func=AF.Exp, scale=SCALE, bias=nln512[:],
                            accum_out=racc[:, h, j:j + 1])

            def vt_closures(b):
                racc, rrec, v_bf = st[b]["racc"], st[b]["rrec"], st[b]["v"]
                vtb = attn_pool.tile([P, KC, D], BF16, tag="vtb")
                st[b]["vtb"] = vtb

                def per_tt(tt):
                    def go():
                        # racc holds S/512 (ln512 folded into the exp bias),
                        # so rrec = 512/S directly
                        nc.vector.reciprocal(out=rrec[:, :, tt],
                                             in_=racc[:, :, tt])
                        base = rrec[:, 0, tt]
                        r_bc = bass.AP(tensor=base.tensor, offset=base.offset,
                                       ap=[base.ap[0], [KC, H], [0, DH]])
                        vsl = v_bf[:, tt, :]
                        v3 = bass.AP(tensor=vsl.tensor, offset=vsl.offset,
                                     ap=[vsl.ap[0], [DH, H], [1, DH]])
                        osl = vtb[:, tt, :]
                        o3 = bass.AP(tensor=osl.tensor, offset=osl.offset,
                                     ap=[osl.ap[0], [DH, H], [1, DH]])
                        if tt >= 2 and CFG["vt_tail_pool"]:
                            nc.gpsimd.tensor_tensor(out=o3, in0=v3, in1=r_bc,
                                                    op=ALU.mult)
                        else:
                            eng(CFG["conv"]["vt"]).tensor_tensor(
                                out=o3, in0=v3, in1=r_bc, op=ALU.mult)
                    return go
                return [per_tt(tt) for tt in range(KC)]

            def apply_closures(b):
                e_t = st[b]["e"]
                attnT8 = attn_pool.tile([P, MC, N], F8, tag="attnT8")
                st[b]["attnT8"] = attnT8

                def per_u(u):
                    def go():
                        vtb = st[b]["vtb"]
                        ps = psA.tile([P, 2, N], F32, tag="psA")
                        for hl in range(2):
                            hp = 2 * u + hl
                            for hh in range(2):
                                h = 2 * hp + hh
                                po = 64 * hh
                                for j in range(KC):
                                    nc.tensor.matmul(
                                        ps[po:po + 64, hl, :],
                                        lhsT=vtb[:, j, 64 * h:64 * h + 64],
                                        rhs=e_t[h][:, j, :],
                                        start=(j == 0), stop=(j == KC - 1),
                                        tile_position=(0, po))
                        conv("attnT", attnT8[:, 2 * u:2 * u + 2, :], ps[:],
                             32.0, e="a" if b == BLOC - 1 else None)
                    return go
                return [per_u(0), per_u(1)]

            def ffn_closures(b):
                qt_b = st[b]["qt"]
                outbf = ffn_pool.tile([P, MC, N], BF16, tag="outbf")
                h1bf = ffn_pool.tile([P, MC, N], BF16, tag="h1bf")

                def wo_u(u):
                    def go():
                        attnT8 = st[b]["attnT8"]
                        ps = psA.tile([P, 2, N], F32, tag="psA")
                        for ml in range(2):
                            m = 2 * u + ml
                            first = True
                            for t in range(2):
                                for wname in ("wo8", "rwo8"):
                                    nc.tensor.matmul(
                                        ps[:, ml, :],
                                        lhsT=w_sb[wname][:, 2 * t:2 * t + 2,
                                                         m * P:(m + 1) * P],
                                        rhs=attnT8[:, 2 * t:2 * t + 2, :],
                                        start=first, stop=False,
                                        perf_mode=DR)
                                    first = False
                            nc.tensor.matmul(
                                ps[:, ml, :], lhsT=id2k[:],
                                rhs=qt_b[:, m, :], start=False, stop=True)
                        if with_bias:
                            for ml in range(2):
                                m = 2 * u + ml
                                eng(CFG["conv"]["outbf"]).tensor_scalar(
                                    out=outbf[:, m, :], in0=ps[:, ml, :],
                                    scalar1=1.0 / (16.0 * SW * 2.0),
                                    scalar2=b_sb["bo"][:, m:m + 1],
                                    op0=ALU.mult, op1=ALU.add)
                        else:
                            conv("outbf", outbf[:, 2 * u:2 * u + 2, :],
                                 ps[:], 1.0 / (16.0 * SW * 2.0),
                                 e="a" if b == BLOC - 1 else None)
                    return go

                def w1_u(u):
                    def go():
                        ps = psA.tile([P, 2, N], F32, tag="psA")
                        for ml in range(2):
                            m = 2 * u + ml
                            for kc in range(KC):
                                nc.tensor.matmul(
                                    ps[:, ml, :],
                                    lhsT=w_sb["w1bf"][:, kc,
                                                      m * P:(m + 1) * P],
                                    rhs=outbf[:, kc, :],
                                    start=(kc == 0), stop=(kc == KC - 1))
                        if with_bias:
                            for ml in range(2):
                                m = 2 * u + ml
                                eng(CFG["conv"]["h1"]).tensor_scalar(
                                    out=h1bf[:, m, :], in0=ps[:, ml, :],
                                    scalar1=b_sb["b1"][:, m:m + 1],
                                    scalar2=0.0, op0=ALU.add, op1=ALU.max)
                        else:
                            conv("h1", h1bf[:, 2 * u:2 * u + 2, :], ps[:],
                                 1.0, relu=True,
                                 e="a" if b == BLOC - 1 else None)
                    return go

                def w2_u(u):
                    def go():
                        ps = psA.tile([P, 2, N], F32, tag="psA")
                        for ml in range(2):
                            m = 2 * u + ml
                            for kc in range(KC):
                                nc.tensor.matmul(
                                    ps[:, ml, :],
                                    lhsT=w_sb["w2bf"][:, kc,
                                                      m * P:(m + 1) * P],
                                    rhs=h1bf[:, kc, :],
                                    start=(kc == 0), stop=False)
                            nc.tensor.matmul(
                                ps[:, ml, :], lhsT=id1[:],
                                rhs=outbf[:, m, :], start=False, stop=True)
                        fin = fin_pool.tile([P, 2, N], F32, tag="fin")
                        if with_bias:
                            for ml in range(2):
                                m = 2 * u + ml
                                nc.vector.tensor_scalar(
                                    out=fin[:, ml, :], in0=ps[:, ml, :],
                                    scalar1=b_sb["b2"][:, m:m + 1],
                                    scalar2=None, op0=ALU.add)
                        else:
                            conv("fin", fin[:], ps[:], 1.0)
                        nc.sync.dma_start(
                            out=outT_v[b][:, 2 * u:2 * u + 2, :], in_=fin[:])
                    return go

                return [wo_u(0), wo_u(1), w1_u(0), w1_u(1), w2_u(0), w2_u(1)]

            # ---- interleaved software pipeline --------------------------
            # stage s: proj(s) | scores+exp(s-1) | vt/apply/ffn(s-2).
            # Per-head scores+exp emissions are round-robined with the other
            # stages' chunks so the in-order PE queue never camps on the
            # ACT-bound softmax chain.
            for b in range(1):
                pass
            # batch 0 inputs load up-front (qt0/q80/k80 already DMA'd)
            st[0] = {"qt": qt0, "q8": q80, "k8": k80}
            deferred = []
            for s in range(BLOC + 2):
                others = list(deferred)
                deferred = []
                if s < BLOC and s > 0:
                    others += dma_in(s)
                if s >= 2:
                    others += apply_closures(s - 2)
                    fc = ffn_closures(s - 2)
                    if s == BLOC:
                        # hold back half the ffn so the pipeline drain has
                        # work to overlap with the last batch's apply
                        others += fc[:2]
                        deferred = fc[2:]
                    else:
                        others += fc
                if s < BLOC:
                    others += proj_closures(s)
                if 1 <= s <= BLOC:
                    b_sc = s - 1
                    scores_prep(b_sc)
                    k = 0
                    for i_h in range(H):
                        scores_exp(b_sc, i_h, (0, 1, 2, 3))
                        take = ((len(others) * (i_h + 1)) // H
                                - (len(others) * i_h) // H)
                        for _ in range(take):
                            others[k]()
                            k += 1
                    while k < len(others):
                        others[k]()
                        k += 1
                    # 1/S + v-fold as soon as all sums are in
                    for c in vt_closures(b_sc):
                        c()
                else:
                    for c in others:
                        c()

    nc.compile()
    return nc


def kernel(Q, K, Wq, bq, Wk, bk, Wv, bv, Wo, bo, W1, b1, W2, b2):
    Q = np.asarray(Q, dtype=np.float32)
    K = np.asarray(K, dtype=np.float32)

    biases = {nm: np.asarray(v, np.float32) for nm, v in
              (("bq", bq), ("bk", bk), ("bv", bv),
               ("bo", bo), ("b1", b1), ("b2", b2))}
    with_bias = any(np.any(v) for v in biases.values())

    key = ("nc", with_bias)
    if key not in _CACHE:
        _CACHE[key] = _build_program(with_bias)
    nc = _CACHE[key]

    wqT = np.asarray(Wq, np.float32).T * SW
    wkT = np.asarray(Wk, np.float32).T * SW
    woT = np.asarray(Wo, np.float32).T * SW
    wo8 = _q8(woT)
    common = {
        "wq8": np.ascontiguousarray(_q8(wqT)),
        "wk8": np.ascontiguousarray(_q8(wkT)),
        "wv8": np.ascontiguousarray(_q8(np.asarray(Wv, np.float32).T * SW)),
        "wo8": np.ascontiguousarray(wo8),
        "rwo8": np.ascontiguousarray(_q8(woT - wo8.astype(np.float32))),
        "w1bf": np.ascontiguousarray(
            np.asarray(W1, np.float32).T.astype(NP_BF)),
        "w2bf": np.ascontiguousarray(
            np.asarray(W2, np.float32).T.astype(NP_BF)),
        "ident1": np.eye(P, dtype=np.float32).astype(NP_BF),
        "ident2k": (np.eye(P, dtype=np.float32) * 2048.0).astype(NP_BF),
    }
    if with_bias:
        common.update({
            "bq": biases["bq"], "bk": biases["bk"],
            "bv": biases["bv"], "bo": biases["bo"],
            "b1": biases["b1"], "b2": biases["b2"],
        })
    in_maps = []
    for c in range(NCORES):
        sl = slice(c * BLOC, (c + 1) * BLOC)
        qT = np.ascontiguousarray(Q[sl].transpose(0, 2, 1))
        kT = np.ascontiguousarray(K[sl].transpose(0, 2, 1))
        in_maps.append({
            "qtbf": qT.astype(NP_BF),
            "qT8": _q8(qT),
            "kT8": _q8(kT),
            **common,
        })

    trace = bool(int(os.environ.get("KERNEL_TRACE", "0")))
    res = run_bass_kernel_spmd(nc, in_maps, core_ids=list(range(NCORES)),
                               trace=trace)
    if trace and res.exec_time_ns is not None:
        print(f"HW exec time: {res.exec_time_ns} ns")

    out = np.empty((B, N, D), np.float32)
    for c in range(NCORES):
        out[c * BLOC:(c + 1) * BLOC] = res.results[c]["outT"].transpose(0, 2, 1)
    return out


# revision 45
# speedup vs baseline: 1.0086x; 1.0051x over previous
"""Trainium2 Bass kernel for nn_MAB (dense transformer block).

Reference (B=32, N=512, D=512, H=8, dh=64):
    q = Q@Wq.T+bq  k = K@Wk.T+bk  v = K@Wv.T+bv
    scores = einsum("bqhd,bkhd->bhqk", q, k) / sqrt(512)
    A = softmax(scores, axis=2)            # over the QUERY axis
    attn = einsum("bhqk,bkhd->bqhd", A, v).reshape(B, N, D)
    out = Q + attn @ Wo.T + bo
    ffn = relu(out @ W1.T + b1) @ W2.T + b2
    return out + ffn

Strategy: data-parallel over batch (8 cores x 4 batches, no collectives).

Matmul engine plan (fp8 = e4m3 with the DoubleRow perf mode, two
128-partition contraction tiles per instruction at 0.5 cycles/row; fp8
matmuls are only ISA-legal as full tiles at PE position (0,0)):
  - q/k/v projections: fp8 DR, plain quantized operands.
  - scores: fp8 DR via a zero-tile trick -- each head's 64 features are
    relocated (SBUF->SBUF DMA on the idle DMA queues) to rows 0:64 of a
    per-head chunk of a [128, 9, N] "z" tile whose other rows/chunk-8
    are zeroed once per buffer; DR tile0 contracts the head against
    zero-padded rows and tile1 multiplies the all-zero chunk.
  - Wo: fp8 DR with an extra fp8 residual-correction matmul (plain fp8
    weight quantization alone exceeds the error budget).
  - attention apply + W1/W2: bf16 (fp8 on E/out/h1 is wasteful or too
    lossy; bf16 apply reuses the proven row-packed head-pair layout).
  - residual adds (+Q, +out) ride the PSUM accumulation via identity
    stationary matmuls, removing two elementwise passes.
Weights are pre-scaled x64 on the host so fp8 quantization stays out of
the subnormal range; compensating factors fold into the PSUM->SBUF
conversion ops, and ln(512) is folded into the exp bias so the softmax
normalizer needs no separate scaling pass.

Softmax over the query axis: scores^T tiles ([k, q]) -> ACT exp; the
per-k sums S over q come from the ACT accumulator for most (head,chunk)
tiles and from DVE tensor_reduce for n_dve_chunks of them (engine
balance knob; GPSIMD cannot reduce the free axis or read PSUM at all).
1/S is folded into v (64x fewer elements than E).

Emission is a 4-deep software pipeline with closure-level interleaving:
proj(b) | scores+exp(b-1) (+1/S+v-fold tail) | apply+ffn(b-2), with the
per-head scores+exp emissions round-robined against the other stages'
chunks so the in-order PE queue never camps on the ACT-bound softmax
chain.  PSUM: psA [128,2,512]x2 (proj/ffn/apply), psS1 [128,512]x4
(scores->exp ping-pong).
"""

import math
import os
import sys

import numpy as np
import ml_dtypes

sys.path.insert(0, "/opt/trn_rl_repo")

import concourse.bass as bass  # noqa: E402
import concourse.tile as tile  # noqa: E402
from concourse import bacc  # noqa: E402
from concourse import mybir  # noqa: E402
from concourse.bass_utils import run_bass_kernel_spmd  # noqa: E402

F32 = mybir.dt.float32
BF16 = mybir.dt.bfloat16
F8 = mybir.dt.float8e4
AF = mybir.ActivationFunctionType
ALU = mybir.AluOpType
DR = mybir.MatmulPerfMode.DoubleRow

NP_F8 = ml_dtypes.float8_e4m3
NP_BF = ml_dtypes.bfloat16

B, N, D, H = 32, 512, 512, 8
DH = D // H
NCORES = 8
BLOC = B // NCORES
SCALE = 1.0 / math.sqrt(512.0)
P = 128
KC = D // P  # 4 contraction chunks
MC = D // P  # 4 output-feature chunks
SW = 64.0  # host-side weight pre-scale before fp8 quantization
_NEG_LN512 = -math.log(512.0)

_CACHE = {}

# --- engine-balance knobs -------------------------------------------------
# conv engines: 'v' = DVE, 'g' = GPSIMD/Pool, 'a' = ACT (tensor_scalar-able
# conversions only).  reduce_eng maps reduce index 0..len-1 -> engine.
CFG = {
    # heads whose S comes from the ACT accumulator (the rest are batched
    # exps + DVE tensor_reduce; GPSIMD cannot reduce the free axis at all)
    "accum_heads": (0, 1, 2, 3, 4, 5, 6, 7),
    "n_dve_chunks": 10,
    "n_batched_heads": 0,  # of 32 (head,chunk) exps: no-accum + DVE reduce
    "vt_tail_pool": False,  # v-fold chunks 2-3 on the otherwise-idle Pool
    # note: GPSIMD ("g") cannot read PSUM; psum-reading convs must be v/a
    "conv": {
        "qh": "v", "kh": "v", "v": "v", "vt": "v",
        "attnT": "v", "outbf": "v", "h1": "v", "fin": "a",
    },
}


def _q8(x):
    return np.asarray(x, np.float32).astype(NP_F8)


def _scores_perm():
    """Column permutation for Wq/Wk: chunk m holds
    m=0: heads 0-3 feats 0-31 | m=1: heads 4-7 feats 0-31
    m=2: heads 0-3 feats 32-63 | m=3: heads 4-7 feats 32-63
    (each head's two 32-feature halves share partitions 32a..32a+31)."""
    perm = np.empty(D, np.int64)
    for m in range(4):
        for a in range(4):
            for f in range(32):
                h = 4 * (m & 1) + a
                perm[128 * m + 32 * a + f] = 64 * h + 32 * (m >> 1) + f
    return perm


def _build_program(with_bias):
    nc = bacc.Bacc("TRN2", target_bir_lowering=False, debug=False,
                   num_devices=NCORES)

    qt_d = nc.dram_tensor("qtbf", [BLOC, D, N], BF16,
                          kind="ExternalInput").ap()
    id1_d = nc.dram_tensor("ident1", [P, P], BF16, kind="ExternalInput").ap()
    id2k_d = nc.dram_tensor("ident2k", [P, P], BF16,
                            kind="ExternalInput").ap()
    qT8_d = nc.dram_tensor("qT8", [BLOC, D, N], F8, kind="ExternalInput").ap()
    kT8_d = nc.dram_tensor("kT8", [BLOC, D, N], F8, kind="ExternalInput").ap()
    w_d = {}
    for nm in ("wq8", "wk8", "wv8", "wo8", "rwo8"):
        w_d[nm] = nc.dram_tensor(nm, [D, D], F8, kind="ExternalInput").ap()
    for nm in ("w1bf", "w2bf"):
        w_d[nm] = nc.dram_tensor(nm, [D, D], BF16, kind="ExternalInput").ap()
    b_d = {}
    if with_bias:
        for nm in ("bq", "bk", "bv", "bo", "b1", "b2"):
            b_d[nm] = nc.dram_tensor(nm, [D], F32, kind="ExternalInput").ap()
    outT_d = nc.dram_tensor("outT", [BLOC, D, N], F32,
                            kind="ExternalOutput").ap()

    qt_v = qt_d.rearrange("b (o p) t -> b p o t", p=P)
    qT8_v = qT8_d.rearrange("b (o p) t -> b p o t", p=P)
    kT8_v = kT8_d.rearrange("b (o p) t -> b p o t", p=P)
    outT_v = outT_d.rearrange("b (o p) t -> b p o t", p=P)
    w_v = {k: v.rearrange("(o p) n -> p o n", p=P) for k, v in w_d.items()}
    b_v = {k: v.rearrange("(o p) -> p o", p=P) for k, v in b_d.items()}

    accum_heads = set(CFG["accum_heads"])

    with tile.TileContext(nc) as tc:
        with (
            tc.tile_pool(name="weights", bufs=1) as wpool,
            tc.tile_pool(name="qt", bufs=3) as qt_pool,
            tc.tile_pool(name="in8", bufs=2) as in8_pool,
            tc.tile_pool(name="proj", bufs=3) as proj_pool,
            tc.tile_pool(name="z8", bufs=2) as z8_pool,
            tc.tile_pool(name="e8", bufs=17) as e8_pool,
            tc.tile_pool(name="rsum", bufs=2) as rsum_pool,
            tc.tile_pool(name="attn", bufs=2) as attn_pool,
            tc.tile_pool(name="ffn", bufs=2) as ffn_pool,
            tc.tile_pool(name="fin", bufs=4) as fin_pool,
            tc.tile_pool(name="psA", bufs=2, space="PSUM") as psA,
            tc.tile_pool(name="psS1", bufs=4, space="PSUM") as psS1,
        ):
            def eng(key):
                return {"v": nc.vector, "g": nc.gpsimd, "a": nc.scalar}[key]

            def conv(key, out, in0, scale, relu=False, e=None):
                """out = [relu](in0 * scale) on the configured engine."""
                e = e or CFG["conv"][key]
                if e == "a":
                    nc.scalar.activation(
                        out=out, in_=in0,
                        func=AF.Relu if relu else AF.Identity, scale=scale)
                elif relu:
                    eng(e).tensor_scalar(out=out, in0=in0, scalar1=scale,
                                         scalar2=0.0, op0=ALU.mult,
                                         op1=ALU.max)
                else:
                    eng(e).tensor_scalar(out=out, in0=in0, scalar1=scale,
                                         scalar2=None, op0=ALU.mult)

            # ---- resident weights ---------------------------------------
            w_sb = {}
            for nm in ("wq8", "wk8", "wv8", "wo8", "rwo8"):
                w_sb[nm] = wpool.tile([P, KC, D], F8, tag=f"w_{nm}",
                                      name=f"w_{nm}")
            for nm in ("w1bf", "w2bf"):
                w_sb[nm] = wpool.tile([P, KC, D], BF16, tag=f"w_{nm}",
                                      name=f"w_{nm}")
            qt0 = qt_pool.tile([P, KC, N], BF16, tag="qt", name="qt0")
            nln512 = wpool.tile([P, 1], F32, tag="nln512")
            nc.vector.memset(nln512[:], _NEG_LN512)
            id1 = wpool.tile([P, P], BF16, tag="id1")
            id2k = wpool.tile([P, P], BF16, tag="id2k")
            nc.sync.dma_start(out=id1[:], in_=id1_d)
            nc.sync.dma_start(out=id2k[:], in_=id2k_d)
            q80 = in8_pool.tile([P, KC, N], F8, tag="q8", name="q80")
            k80 = in8_pool.tile([P, KC, N], F8, tag="k8", name="k80")
            # initial loads fan out across four DGE queues so the first
            # projection's operands arrive as early as possible
            nc.sync.dma_start(out=w_sb["wq8"][:], in_=w_v["wq8"])
            nc.gpsimd.dma_start(out=q80[:], in_=qT8_v[0])
            nc.scalar.dma_start(out=w_sb["wk8"][:], in_=w_v["wk8"])
            nc.gpsimd.dma_start(out=k80[:], in_=kT8_v[0])
            nc.gpsimd.dma_start(out=w_sb["wv8"][:], in_=w_v["wv8"])
            nc.sync.dma_start(out=qt0[:], in_=qt_v[0])
            # PE warm-up: harmless matmuls on the identity tile keep the
            # p-state ramp running while the loads land
            wu = psS1.tile([P, N], F32, tag="psS1", name="warmup")
            for _ in range(10):
                nc.tensor.matmul(wu[:, 0:P], lhsT=id1[:], rhs=id1[:],
                                 start=True, stop=True)

            b_sb = {}
            bv_bc = None
            if with_bias:
                for nm in ("bq", "bk", "bo", "b1", "b2"):
                    b_sb[nm] = wpool.tile([P, MC], F32, tag=f"b_{nm}",
                                          name=f"b_{nm}")
                    nc.sync.dma_start(out=b_sb[nm][:], in_=b_v[nm])
                bv_bc = wpool.tile([P, D], F32, tag="bv_bc")
                bv_src = bass.AP(tensor=b_d["bv"].tensor,
                                 offset=b_d["bv"].offset,
                                 ap=[[0, P], *b_d["bv"].ap])
                nc.sync.dma_start(out=bv_bc[:], in_=bv_src)

            st = {}

            def dma_in(b):
                qt_b = qt_pool.tile([P, KC, N], BF16, tag="qt")
                q8_b = in8_pool.tile([P, KC, N], F8, tag="q8")
                k8_b = in8_pool.tile([P, KC, N], F8, tag="k8")
                st[b] = {"qt": qt_b, "q8": q8_b, "k8": k8_b}

                def go():
                    nc.sync.dma_start(out=qt_b[:], in_=qt_v[b])
                    nc.sync.dma_start(out=q8_b[:], in_=qT8_v[b])
                    nc.sync.dma_start(out=k8_b[:], in_=kT8_v[b])
                return [go]

            def proj_closures(b):
                qt_b, q8_b, k8_b = st[b]["qt"], st[b]["q8"], st[b]["k8"]
                qhb = proj_pool.tile([P, MC, N], F8, tag="qhb")
                khb = proj_pool.tile([P, MC, N], F8, tag="khb")
                v_bf = proj_pool.tile([P, KC, D], BF16, tag="v")
                qz = z8_pool.tile([P, 9, N], F8, tag="qz", name=f"qz_{b}")
                kz = z8_pool.tile([P, 9, N], F8, tag="kz", name=f"kz_{b}")
                st[b].update({"qhb": qhb, "khb": khb, "v": v_bf,
                              "qz": qz, "kz": kz})

                def lin_u(dst, key, wname, rhs, bias, u):
                    def go():
                        ps = psA.tile([P, 2, N], F32, tag="psA")
                        for ml in range(2):
                            m = 2 * u + ml
                            for t in range(2):
                                nc.tensor.matmul(
                                    ps[:, ml, :],
                                    lhsT=w_sb[wname][:, 2 * t:2 * t + 2,
                                                     m * P:(m + 1) * P],
                                    rhs=rhs[:, 2 * t:2 * t + 2, :],
                                    start=(t == 0), stop=(t == 1),
                                    perf_mode=DR)
                        if with_bias and bias is not None:
                            for ml in range(2):
                                m = 2 * u + ml
                                eng(CFG["conv"][key]).tensor_scalar(
                                    out=dst[:, m, :], in0=ps[:, ml, :],
                                    scalar1=1.0 / SW,
                                    scalar2=b_sb[bias][:, m:m + 1],
                                    op0=ALU.mult, op1=ALU.add)
                        else:
                            conv(key, dst[:, 2 * u:2 * u + 2, :], ps[:],
                                 1.0 / SW)
                    return go

                def v_u(u):
                    def go():
                        ps = psA.tile([P, 2, N], F32, tag="psA")
                        for tl in range(2):
                            tt = 2 * u + tl
                            for t in range(2):
                                nc.tensor.matmul(
                                    ps[:, tl, :],
                                    lhsT=k8_b[:, 2 * t:2 * t + 2,
                                              tt * P:(tt + 1) * P],
                                    rhs=w_sb["wv8"][:, 2 * t:2 * t + 2, :],
                                    start=(t == 0), stop=(t == 1),
                                    perf_mode=DR)
                        if with_bias:
                            for tl in range(2):
                                tt = 2 * u + tl
                                eng(CFG["conv"]["v"]).scalar_tensor_tensor(
                                    out=v_bf[:, tt, :], in0=ps[:, tl, :],
                                    scalar=1.0 / SW, in1=bv_bc[:],
                                    op0=ALU.mult, op1=ALU.add)
                        else:
                            conv("v", v_bf[:, 2 * u:2 * u + 2, :], ps[:],
                                 1.0 / SW)
                    return go

                def zbuild(u):
                    # per-head relocation: head h's 64 q/k features ->
                    # rows 0:64 of z-chunk h; chunk 8 and rows 64:128 of
                    # qz stay zero (memset once per pool buffer) so the
                    # DoubleRow zero-tile trick contracts only head h.
                    # u=0 covers heads 0-3 (proj chunks 0-1), u=1 heads 4-7.
                    def go():
                        if b < 2 and u == 0:
                            nc.gpsimd.memset(qz[64:P, :, :], 0.0)
                            nc.gpsimd.memset(qz[0:64, 8, :], 0.0)
                            nc.gpsimd.memset(kz[64:P, :, :], 0.0)
                            nc.gpsimd.memset(kz[0:64, 8, :], 0.0)
                        cs = slice(2 * u, 2 * u + 2)
                        for src_t, dst_t in ((qhb, qz), (khb, kz)):
                            nc.sync.dma_start(
                                out=dst_t[0:64, 4 * u:4 * u + 4:2, :],
                                in_=src_t[0:64, cs, :])
                            nc.sync.dma_start(
                                out=dst_t[0:64, 4 * u + 1:4 * u + 4:2, :],
                                in_=src_t[64:P, cs, :])
                    return go

                out = [lin_u(qhb, "qh", "wq8", q8_b, "bq", 0),
                       lin_u(khb, "kh", "wk8", k8_b, "bk", 0),
                       zbuild(0),
                       lin_u(qhb, "qh", "wq8", q8_b, "bq", 1),
                       lin_u(khb, "kh", "wk8", k8_b, "bk", 1),
                       zbuild(1),
                       v_u(0), v_u(1)]
                if b == 0:
                    def defer_w():
                        for nm in ("wo8", "rwo8", "w1bf", "w2bf"):
                            nc.sync.dma_start(out=w_sb[nm][:], in_=w_v[nm])
                    out.append(defer_w)
                return out

            def head_sel(h):
                # head h's 64 features: partitions 64*(h%2).. of chunk h//2
                return slice(64 * (h % 2), 64 * (h % 2) + 64), h // 2

            def scores_prep(b):
                st[b]["racc"] = rsum_pool.tile([P, H, KC], F32, tag="racc",
                                               name=f"racc_{b}")
                st[b]["rrec"] = rsum_pool.tile([P, H, KC], F32, tag="rrec",
                                               name=f"rrec_{b}")
                st[b]["e"] = {}

            def scores_exp(b, h, js):
                qz, kz, racc = st[b]["qz"], st[b]["kz"], st[b]["racc"]
                if js[0] == 0:
                    e_h = e8_pool.tile([P, KC, N], BF16, tag="e8",
                                       name=f"e_{b}_{h}")
                    st[b]["e"][h] = e_h
                else:
                    e_h = st[b]["e"][h]
                zsl = slice(h, 9, 8 - h)  # chunks {h, 8}; 8 is all-zero in qz
                if h < CFG["n_batched_heads"]:
                    # S on DVE: batch the exps [128,1024] to amortize init
                    for cp in (js[0] // 2, js[2] // 2) if len(js) == 4 \
                            else (js[0] // 2,):
                        ps = psA.tile([P, 2, N], F32, tag="psA")
                        for jj in range(2):
                            j = 2 * cp + jj
                            nc.tensor.matmul(
                                ps[:, jj, :],
                                lhsT=kz[:, zsl, j * P:(j + 1) * P],
                                rhs=qz[:, zsl, :], start=True, stop=True,
                                perf_mode=DR)
                        nc.scalar.activation(
                            out=e_h[:, 2 * cp:2 * cp + 2, :], in_=ps[:],
                            func=AF.Exp, scale=SCALE, bias=nln512[:])
                        for jj in range(2):
                            j = 2 * cp + jj
                            nc.vector.tensor_reduce(
                                out=racc[:, h, j:j + 1], in_=e_h[:, j, :],
                                axis=mybir.AxisListType.X, op=ALU.add)
                    return
                for j in js:
                    ps = psS1.tile([P, N], F32, tag="psS1")
                    nc.tensor.matmul(
                        ps[:], lhsT=kz[:, zsl, j * P:(j + 1) * P],
                        rhs=qz[:, zsl, :], start=True, stop=True,
                        perf_mode=DR)
                    if j < 2 and h < 6:  # DVE-reduced: early chunks of heads 0-5
                        nc.scalar.activation(
                            out=e_h[:, j, :], in_=ps[:],
                            func=AF.Exp, scale=SCALE, bias=nln512[:])
                        nc.vector.tensor_reduce(
                            out=racc[:, h, j:j + 1], in_=e_h[:, j, :],
                            axis=mybir.AxisListType.X, op=ALU.add)
                    else:
                        nc.scalar.activation(
                            out=e_h[:, j, :], in_=ps[:],
                            func=AF.Exp, scale=SCALE, bias=nln512[:],
                            accum_out=racc[:, h, j:j + 1])

            def vt_closures(b):
                racc, rrec, v_bf = st[b]["racc"], st[b]["rrec"], st[b]["v"]
                vtb = attn_pool.tile([P, KC, D], BF16, tag="vtb")
                st[b]["vtb"] = vtb

                def per_tt(tt):
                    def go():
                        # racc holds S/512 (ln512 folded into the exp bias),
                        # so rrec = 512/S directly
                        nc.vector.reciprocal(out=rrec[:, :, tt],
                                             in_=racc[:, :, tt])
                        base = rrec[:, 0, tt]
                        r_bc = bass.AP(tensor=base.tensor, offset=base.offset,
                                       ap=[base.ap[0], [KC, H], [0, DH]])
                        vsl = v_bf[:, tt, :]
                        v3 = bass.AP(tensor=vsl.tensor, offset=vsl.offset,
                                     ap=[vsl.ap[0], [DH, H], [1, DH]])
                        osl = vtb[:, tt, :]
                        o3 = bass.AP(tensor=osl.tensor, offset=osl.offset,
                                     ap=[osl.ap[0], [DH, H], [1, DH]])
                        if tt >= 2 and CFG["vt_tail_pool"]:
                            nc.gpsimd.tensor_tensor(out=o3, in0=v3, in1=r_bc,
                                                    op=ALU.mult)
                        else:
                            eng(CFG["conv"]["vt"]).tensor_tensor(
                                out=o3, in0=v3, in1=r_bc, op=ALU.mult)
                    return go
                return [per_tt(tt) for tt in range(KC)]

            def apply_closures(b):
                e_t = st[b]["e"]
                attnT8 = attn_pool.tile([P, MC, N], F8, tag="attnT8")
                st[b]["attnT8"] = attnT8

                def per_u(u):
                    def go():
                        vtb = st[b]["vtb"]
                        ps = psA.tile([P, 2, N], F32, tag="psA")
                        for hl in range(2):
                            hp = 2 * u + hl
                            for hh in range(2):
                                h = 2 * hp + hh
                                po = 64 * hh
                                for j in range(KC):
                                    nc.tensor.matmul(
                                        ps[po:po + 64, hl, :],
                                        lhsT=vtb[:, j, 64 * h:64 * h + 64],
                                        rhs=e_t[h][:, j, :],
                                        start=(j == 0), stop=(j == KC - 1),
                                        tile_position=(0, po))
                        conv("attnT", attnT8[:, 2 * u:2 * u + 2, :], ps[:],
                             32.0, e="a" if b == BLOC - 1 else None)
                    return go
                return [per_u(0), per_u(1)]

            def ffn_closures(b):
                qt_b = st[b]["qt"]
                outbf = ffn_pool.tile([P, MC, N], BF16, tag="outbf")
                h1bf = ffn_pool.tile([P, MC, N], BF16, tag="h1bf")

                def wo_u(u):
                    def go():
                        attnT8 = st[b]["attnT8"]
                        ps = psA.tile([P, 2, N], F32, tag="psA")
                        for ml in range(2):
                            m = 2 * u + ml
                            first = True
                            for t in range(2):
                                for wname in ("wo8", "rwo8"):
                                    nc.tensor.matmul(
                                        ps[:, ml, :],
                                        lhsT=w_sb[wname][:, 2 * t:2 * t + 2,
                                                         m * P:(m + 1) * P],
                                        rhs=attnT8[:, 2 * t:2 * t + 2, :],
                                        start=first, stop=False,
                                        perf_mode=DR)
                                    first = False
                            nc.tensor.matmul(
                                ps[:, ml, :], lhsT=id2k[:],
                                rhs=qt_b[:, m, :], start=False, stop=True)
                        if with_bias:
                            for ml in range(2):
                                m = 2 * u + ml
                                eng(CFG["conv"]["outbf"]).tensor_scalar(
                                    out=outbf[:, m, :], in0=ps[:, ml, :],
                                    scalar1=1.0 / (16.0 * SW * 2.0),
                                    scalar2=b_sb["bo"][:, m:m + 1],
                                    op0=ALU.mult, op1=ALU.add)
                        else:
                            conv("outbf", outbf[:, 2 * u:2 * u + 2, :],
                                 ps[:], 1.0 / (16.0 * SW * 2.0),
                                 e="a" if b == BLOC - 1 else None)
                    return go

                def w1_u(u):
                    def go():
                        ps = psA.tile([P, 2, N], F32, tag="psA")
                        for ml in range(2):
                            m = 2 * u + ml
                            for kc in range(KC):
                                nc.tensor.matmul(
                                    ps[:, ml, :],
                                    lhsT=w_sb["w1bf"][:, kc,
                                                      m * P:(m + 1) * P],
                                    rhs=outbf[:, kc, :],
                                    start=(kc == 0), stop=(kc == KC - 1))
                        if with_bias:
                            for ml in range(2):
                                m = 2 * u + ml
                                eng(CFG["conv"]["h1"]).tensor_scalar(
                                    out=h1bf[:, m, :], in0=ps[:, ml, :],
                                    scalar1=b_sb["b1"][:, m:m + 1],
                                    scalar2=0.0, op0=ALU.add, op1=ALU.max)
                        else:
                            conv("h1", h1bf[:, 2 * u:2 * u + 2, :], ps[:],
                                 1.0, relu=True,
                                 e="a" if b == BLOC - 1 else None)
                    return go

                def w2_u(u):
                    def go():
                        ps = psA.tile([P, 2, N], F32, tag="psA")
                        for ml in range(2):
                            m = 2 * u + ml
                            for kc in range(KC):
                                nc.tensor.matmul(
                                    ps[:, ml, :],
                                    lhsT=w_sb["w2bf"][:, kc,
                                                      m * P:(m + 1) * P],
                                    rhs=h1bf[:, kc, :],
                                    start=(kc == 0), stop=False)
                            nc.tensor.matmul(
                                ps[:, ml, :], lhsT=id1[:],
                                rhs=outbf[:, m, :], start=False, stop=True)
                        fin = fin_pool.tile([P, 2, N], F32, tag="fin")
                        if with_bias:
                            for ml in range(2):
                                m = 2 * u + ml
                                nc.vector.tensor_scalar(
                                    out=fin[:, ml, :], in0=ps[:, ml, :],
                                    scalar1=b_sb["b2"][:, m:m + 1],
                                    scalar2=None, op0=ALU.add)
                        else:
                            conv("fin", fin[:], ps[:], 1.0)
                        nc.sync.dma_start(
                            out=outT_v[b][:, 2 * u:2 * u + 2, :], in_=fin[:])
                    return go

                return [wo_u(0), wo_u(1), w1_u(0), w1_u(1), w2_u(0), w2_u(1)]

            # ---- interleaved software pipeline --------------------------
            # stage s: proj(s) | scores+exp(s-1) | vt/apply/ffn(s-2).
            # Per-head scores+exp emissions are round-robined with the other
            # stages' chunks so the in-order PE queue never camps on the
            # ACT-bound softmax chain.
            for b in range(1):
                pass
            # batch 0 inputs load up-front (qt0/q80/k80 already DMA'd)
            st[0] = {"qt": qt0, "q8": q80, "k8": k80}
            deferred = []
            for s in range(BLOC + 2):
                others = list(deferred)
                deferred = []
                if s < BLOC and s > 0:
                    others += dma_in(s)
                if s >= 2:
                    others += apply_closures(s - 2)
                    fc = ffn_closures(s - 2)
                    if s == BLOC:
                        # hold back half the ffn so the pipeline drain has
                        # work to overlap with the last batch's apply
                        others += fc[:2]
                        deferred = fc[2:]
                    else:
                        others += fc
                if s < BLOC:
                    others += proj_closures(s)
                if 1 <= s <= BLOC:
                    b_sc = s - 1
                    scores_prep(b_sc)
                    k = 0
                    for i_h in range(H):
                        scores_exp(b_sc, i_h, (0, 1, 2, 3))
                        take = ((len(others) * (i_h + 1)) // H
                                - (len(others) * i_h) // H)
                        for _ in range(take):
                            others[k]()
                            k += 1
                    while k < len(others):
                        others[k]()
                        k += 1
                    # 1/S + v-fold as soon as all sums are in
                    for c in vt_closures(b_sc):
                        c()
                else:
                    for c in others:
                        c()

    nc.compile()
    return nc


def kernel(Q, K, Wq, bq, Wk, bk, Wv, bv, Wo, bo, W1, b1, W2, b2):
    Q = np.asarray(Q, dtype=np.float32)
    K = np.asarray(K, dtype=np.float32)

    biases = {nm: np.asarray(v, np.float32) for nm, v in
              (("bq", bq), ("bk", bk), ("bv", bv),
               ("bo", bo), ("b1", b1), ("b2", b2))}
    with_bias = any(np.any(v) for v in biases.values())

    key = ("nc", with_bias)
    if key not in _CACHE:
        _CACHE[key] = _build_program(with_bias)
    nc = _CACHE[key]

    wqT = np.asarray(Wq, np.float32).T * SW
    wkT = np.asarray(Wk, np.float32).T * SW
    woT = np.asarray(Wo, np.float32).T * SW
    wo8 = _q8(woT)
    common = {
        "wq8": np.ascontiguousarray(_q8(wqT)),
        "wk8": np.ascontiguousarray(_q8(wkT)),
        "wv8": np.ascontiguousarray(_q8(np.asarray(Wv, np.float32).T * SW)),
        "wo8": np.ascontiguousarray(wo8),
        "rwo8": np.ascontiguousarray(_q8(woT - wo8.astype(np.float32))),
        "w1bf": np.ascontiguousarray(
            np.asarray(W1, np.float32).T.astype(NP_BF)),
        "w2bf": np.ascontiguousarray(
            np.asarray(W2, np.float32).T.astype(NP_BF)),
        "ident1": np.eye(P, dtype=np.float32).astype(NP_BF),
        "ident2k": (np.eye(P, dtype=np.float32) * 2048.0).astype(NP_BF),
    }
    if with_bias:
        common.update({
            "bq": biases["bq"], "bk": biases["bk"],
            "bv": biases["bv"], "bo": biases["bo"],
            "b1": biases["b1"], "b2": biases["b2"],
        })
    in_maps = []
    for c in range(NCORES):
        sl = slice(c * BLOC, (c + 1) * BLOC)
        qT = np.ascontiguousarray(Q[sl].transpose(0, 2, 1))
        kT = np.ascontiguousarray(K[sl].transpose(0, 2, 1))
        in_maps.append({
            "qtbf": qT.astype(NP_BF),
            "qT8": _q8(qT),
            "kT8": _q8(kT),
            **common,
        })

    trace = bool(int(os.environ.get("KERNEL_TRACE", "0")))
    res = run_bass_kernel_spmd(nc, in_maps, core_ids=list(range(NCORES)),
                               trace=trace)
    if trace and res.exec_time_ns is not None:
        print(f"HW exec time: {res.exec_time_ns} ns")

    out = np.empty((B, N, D), np.float32)
    for c in range(NCORES):
        out[c * BLOC:(c + 1) * BLOC] = res.results[c]["outT"].transpose(0, 2, 1)
    return out
